# revision 22
# baseline (speedup 1.0000x reference)
"""Trainium2 Bass kernel for nn_HausdorffDistance (retrieval_knn).

Computes, for each of B*T = 8 independent problems (sharded 1 problem/core
across 8 NeuronCores):
    nn_dist[i] = min_j ||data1[i] - data2[j]||  (N=M=4096, D=3)
    out[b]     = mean over (t, i) of nn_dist

Device-side algorithm (per core):
  d2[i,j] = |a_i|^2 + |b_j|^2 - 2 a_i . b_j computed fully on the
  TensorEngine via a split-bf16 matmul (each f32 value split into 3 bf16
  terms; K=24 rows), accumulated in f32 PSUM.  Because PSUM holds d2 (>= 0,
  no cancellation left), bf16 intermediates are safe downstream.

  PSUM evacuation (the former bottleneck) is spread over three engines:
    V-tiles: DVE TENSOR_TENSOR_REDUCE straight from PSUM (2 elem/cycle).
    G-tiles: GpSimd scalar_tensor_tensor min folds PSUM pairs into bf16
             SBUF; DVE finishes with 4x-mode bf16 TSP + one TTR.
    A-tiles: ScalarEngine copies PSUM->bf16 SBUF; DVE folds with 4x-mode
             TSP cascade + one TTR.
  Host takes sqrt and means (tiny: 4096 values/problem).
"""

import sys

sys.path.insert(0, "/opt/trn_rl_repo")

from contextlib import ExitStack

import ml_dtypes
import numpy as np

import concourse.bass as bass
import concourse.tile as tile
from concourse import mybir
from concourse.bass_utils import run_bass_kernel_spmd
from concourse.tile import ScopedClock

BF16 = ml_dtypes.bfloat16

N = 4096          # points per set
K = 24            # split-matmul contraction rows
N_TILES = 32      # 4096 / 128 i-tiles
UNITS = 4         # j-chunks of 1024 per i-tile
BIG = 3.0e38      # min-reduce init / filler

# Per-UNIT evacuation engine, rotating with period 8 over the global unit
# stream.  Each unit's PSUM banks must be freed within 3 PE unit-times
# (~1281ns at full clock) or the PE stalls and drops out of its fast
# p-state.  The rotation spaces every engine's units at least as far apart
# as its per-unit latency: DVE ttr 658ns, Pool TSP 806ns, Act copy 1038ns.
#   D: DVE tensor_tensor_reduce, PSUM pair -> min column directly.
#   P: GpSimd scalar_tensor_tensor, PSUM pair -> 512 bf16 chunk.
#   A: Act copy, PSUM -> 1024 bf16 chunk.
# P=64/A=48/D=16 units saturates Pool at 94%, Act at 91%, DVE at ~100% --
# the capacity-balance optimum.  Chunks land in a per-tile buffer; a
# deferred DVE ttr folds them to the tile's min column, flushed one per
# unit boundary but never just before a D-unit (it would delay the
# PSUM-freeing ttr past its deadline).
UNIT_PAT = ["A", "P", "A", "P", "P", "A", "P", "D"]


def _patch_tile_drain():
    """Walrus (CoreV3) rejects the TileContext tail Drain when it carries >1
    sem wait ("Too many sync wait commands").  Split the waits across
    preceding SP NOPs, one wait each."""
    if getattr(tile.TileContext, "_drain_patched", False):
        return

    def _drain_and_barrier(self, tick_clock, wait_clock):
        nc = self.nc
        nops = [nc.sync.nop() for _ in range(31)]
        drain_inst = nc.sync.drain()
        wait_clock.add_sem_waits(
            drain_inst.ins, ScopedClock({None: tick_clock.global_clock})
        )
        si = drain_inst.ins.sync_info
        waits = list(si.on_wait or [])
        used = 0
        if len(waits) > 1:
            si.on_wait = waits[:1]
            used = len(waits) - 1
            for k, w in enumerate(waits[1:]):
                nsi = nops[k].ins.sync_info
                if nsi is None:
                    nops[k].ins.sync_info = mybir.SyncInfo(on_wait=[w], on_update=[])
                else:
                    nsi.on_wait = (nsi.on_wait or []) + [w]
        # drop the unused filler NOPs (each costs ~25ns of SP decode at drain)
        for spare in nops[used:]:
            for bb in nc.m.functions[0].blocks:
                if spare.ins in bb.instructions:
                    bb.instructions.remove(spare.ins)
                    break
        nc.all_engine_barrier()
        popped = nc._tile_sem_poison_stack.pop()
        assert popped is self._sem_poison
        nc.clear_and_free_semaphores(list(self.sems.allocated().values()))
        nc.all_engine_barrier()

    tile.TileContext._drain_and_barrier = _drain_and_barrier
    tile.TileContext._drain_patched = True


_NC_CACHE = None


def _split_multi_waits(nc):
    """This walrus build allows only 1 sem wait per instruction.  Hoist extra
    waits onto the nearest preceding same-engine instruction with a free wait
    slot (in-order engines: waiting earlier is strictly more conservative)."""
    eng_handles = {
        mybir.EngineType.PE: nc.tensor,
        mybir.EngineType.DVE: nc.vector,
        mybir.EngineType.Activation: nc.scalar,
        mybir.EngineType.Pool: nc.gpsimd,
        mybir.EngineType.SP: nc.sync,
    }
    for bb in nc.m.functions[0].blocks:
        insts = list(bb.instructions)
        for idx, inst in enumerate(insts):
            si = inst.sync_info
            if not si or not si.on_wait or len(si.on_wait) <= 1:
                continue
            waits = list(si.on_wait)
            extra = waits[1:]
            si.on_wait = waits[:1]
            for w in extra:
                # Materialize a NOP on this engine right before `inst` to
                # carry the extra wait — same program point, so semantics
                # are identical.  (nop() appends to the function's last
                # block; move it into place.)
                nop = eng_handles[inst.engine].nop().ins
                nc.m.functions[0].blocks[-1].instructions.remove(nop)
                bb.instructions.insert(bb.instructions.index(inst), nop)
                nop.sync_info = mybir.SyncInfo(on_wait=[w], on_update=[])


def _build_nc():
    global _NC_CACHE
    if _NC_CACHE is not None:
        return _NC_CACHE
    _patch_tile_drain()

    nc = bass.Bass(
        "TRN2",
        target_bir_lowering=False,
        debug=False,
        enable_asserts=False,
        num_devices=8,
    )
    inp_ap = nc.dram_tensor("inp", [K, 2 * N], mybir.dt.bfloat16, kind="ExternalInput").ap()
    mins_ap = nc.dram_tensor("mins", [128, N_TILES * UNITS], mybir.dt.float32, kind="ExternalOutput").ap()

    f32 = mybir.dt.float32
    bf16 = mybir.dt.bfloat16
    mn = mybir.AluOpType.min

    with tile.TileContext(nc) as tc:
        with ExitStack() as ctx:
            consts = ctx.enter_context(tc.tile_pool(name="consts", bufs=1))
            psum = ctx.enter_context(tc.tile_pool(name="psum", bufs=4, space="PSUM"))
            tbpool = ctx.enter_context(tc.tile_pool(name="tbuf", bufs=3))
            scr = ctx.enter_context(tc.tile_pool(name="scratch", bufs=1))
            outp = ctx.enter_context(tc.tile_pool(name="outp", bufs=1))

            inp_sb = consts.tile([K, 2 * N], bf16)
            # first matmul's operands first (b chunk, then tile-0 a block),
            # then the rest; DMA issue pipelines but transfers serialize, so
            # fine-grained leading chunks start the PE ~1.5us earlier.
            # HWDGE on SP keeps descriptor generation off the compute engines.
            nc.sync.dma_start(inp_sb[:, N : N + 512], inp_ap[:, N : N + 512])
            nc.sync.dma_start(inp_sb[:, 0:128], inp_ap[:, 0:128])
            nc.sync.dma_start(inp_sb[:, N + 512 : N + 1024], inp_ap[:, N + 512 : N + 1024])
            nc.sync.dma_start(inp_sb[:, N + 1024 : 2 * N], inp_ap[:, N + 1024 : 2 * N])
            nc.sync.dma_start(inp_sb[:, 128:N], inp_ap[:, 128:N])

            mins_sb = outp.tile([128, N_TILES * UNITS], f32)
            nc.vector.memset(mins_sb[:], BIG)

            sc = scr.tile([128, 1536], bf16)

            pending = []  # deferred tail reduces: (tb, nbytes_used, col)

            def flush_tail():
                tb_, used, col = pending.pop(0)
                half = used // 2
                nc.vector.tensor_tensor_reduce(
                    out=sc[:, 0:half],
                    in0=tb_[:, 0:half],
                    in1=tb_[:, half:used],
                    scale=1.0,
                    scalar=BIG,
                    op0=mn,
                    op1=mn,
                    accum_out=mins_sb[:, col : col + 1],
                )

            for t in range(N_TILES):
                lw = inp_sb[:, t * 128 : (t + 1) * 128]
                tb = tbpool.tile([128, 3072], bf16, name="tb")
                off = 0
                tail_col = None
                for u in range(UNITS):
                    g = 4 * t + u
                    e = UNIT_PAT[g % 8]
                    pt = psum.tile([128, 1024], f32)
                    for h in range(2):
                        j0 = N + u * 1024 + h * 512
                        nc.tensor.matmul(
                            pt[:, h * 512 : (h + 1) * 512],
                            lw,
                            inp_sb[:, j0 : j0 + 512],
                            start=True,
                            stop=True,
                        )
                    if e == "D":
                        nc.vector.tensor_tensor_reduce(
                            out=sc[:, 0:512],
                            in0=pt[:, 0:512],
                            in1=pt[:, 512:1024],
                            scale=1.0,
                            scalar=BIG,
                            op0=mn,
                            op1=mn,
                            accum_out=mins_sb[:, g : g + 1],
                        )
                    elif e == "P":
                        nc.gpsimd.scalar_tensor_tensor(
                            out=tb[:, off : off + 512],
                            in0=pt[:, 0:512],
                            scalar=BIG,
                            in1=pt[:, 512:1024],
                            op0=mn,
                            op1=mn,
                        )
                        if tail_col is None:
                            tail_col = g
                        off += 512
                    else:  # A
                        nc.scalar.copy(
                            out=tb[:, off : off + 1024],
                            in_=pt[:, 0:1024],
                        )
                        if tail_col is None:
                            tail_col = g
                        off += 1024
                    # flush one deferred tail per unit boundary, but never
                    # right before a D-unit: its PSUM-freeing ttr must not
                    # queue behind a ~1.2-1.7us tail
                    if pending and (g + 1 >= 4 * N_TILES or UNIT_PAT[(g + 1) % 8] != "D"):
                        flush_tail()
                if off:
                    pending.append((tb, off, tail_col))
                if t == 17:
                    # first half of the output is final once tile 15's tail
                    # has flushed; overlap its DMA with the back half
                    nc.sync.dma_start(mins_ap[:, 0:64], mins_sb[:, 0:64])
            while pending:
                flush_tail()
            nc.sync.dma_start(mins_ap[:, 64:128], mins_sb[:, 64:128])

    _split_multi_waits(nc)
    _NC_CACHE = nc
    return nc


def _split3(x):
    """x (f32) -> three bf16 parts whose (f32) sum ~= x to ~2^-27 rel."""
    x = x.astype(np.float32)
    h = x.astype(BF16).astype(np.float32)
    r = x - h
    l = r.astype(BF16).astype(np.float32)
    q = (r - l).astype(BF16).astype(np.float32)
    return h, l, q


def _prep_problem(A, B):
    """Build lhsT [K, N] and rhs [K, N] bf16 rows for d2 = |a|^2+|b|^2-2a.b."""
    b2 = (B.astype(np.float64) ** 2).sum(1).astype(np.float32)
    a2 = (A.astype(np.float64) ** 2).sum(1).astype(np.float32)
    b2h, b2l, b2q = _split3(b2)
    a2h, a2l, a2q = _split3(a2)
    ah, al, aq = _split3(A)
    bh, bl, bq = _split3(B)
    ones = np.ones(N, np.float32)
    lhs_rows = [ones, ones, ones, a2h, a2l, a2q]
    rhs_rows = [b2h, b2l, b2q, ones, ones, ones]
    for d in range(3):
        for a_, b_ in (
            (ah[:, d], -2.0 * bh[:, d]),
            (ah[:, d], -2.0 * bl[:, d]),
            (al[:, d], -2.0 * bh[:, d]),
            (al[:, d], -2.0 * bl[:, d]),
            (ah[:, d], -2.0 * bq[:, d]),
            (aq[:, d], -2.0 * bh[:, d]),
        ):
            lhs_rows.append(a_)
            rhs_rows.append(b_)
    lhsT = np.stack(lhs_rows).astype(BF16)
    rhs = np.stack(rhs_rows).astype(BF16)
    return np.concatenate([lhsT, rhs], axis=1)  # [K, 2N]


def _run(data1, data2, trace=False):
    d1 = np.asarray(data1, dtype=np.float32).reshape(8, N, 3)
    d2 = np.asarray(data2, dtype=np.float32).reshape(8, N, 3)
    in_maps = []
    for p in range(8):
        in_maps.append({"inp": _prep_problem(d1[p], d2[p])})
    nc = _build_nc()
    res = run_bass_kernel_spmd(nc, in_maps, core_ids=list(range(8)), trace=trace)

    out = np.zeros(2, np.float64)
    for p in range(8):
        m = res.results[p]["mins"]          # [128, 128]; cols = (tile, unit)
        m = m.reshape(128, N_TILES, UNITS).min(axis=-1)   # [128, 32]
        mflat = m.T.reshape(N).astype(np.float64)
        dd = np.sqrt(np.maximum(mflat, 0.0))
        out[p // 4] += dd.mean() / 4.0
    return out.astype(np.float32), res


def kernel(data1, data2, dim):
    dim = int(dim)
    if dim > 0:
        data1 = np.swapaxes(np.asarray(data1), 0, dim)
        data2 = np.swapaxes(np.asarray(data2), 0, dim)
    out, _ = _run(data1, data2, trace=False)
    return out


def kernel_traced(data1, data2, dim):
    """test.py entry: returns (output, BassKernelResults) with profiling."""
    dim = int(dim)
    if dim > 0:
        data1 = np.swapaxes(np.asarray(data1), 0, dim)
        data2 = np.swapaxes(np.asarray(data2), 0, dim)
    return _run(data1, data2, trace=True)


# revision 23
# speedup vs baseline: 1.0523x; 1.0523x over previous
"""Trainium2 Bass kernel for nn_HausdorffDistance (retrieval_knn).

Computes, for each of B*T = 8 independent problems (sharded 1 problem/core
across 8 NeuronCores):
    nn_dist[i] = min_j ||data1[i] - data2[j]||  (N=M=4096, D=3)
    out[b]     = mean over (t, i) of nn_dist

Device-side algorithm (per core):
  d2[i,j] = |a_i|^2 + |b_j|^2 - 2 a_i . b_j computed fully on the
  TensorEngine via a split-bf16 matmul (each f32 value split into 3 bf16
  terms; K=24 rows), accumulated in f32 PSUM.  Because PSUM holds d2 (>= 0,
  no cancellation left), bf16 intermediates are safe downstream.

  PSUM evacuation (the former bottleneck) is spread over three engines:
    V-tiles: DVE TENSOR_TENSOR_REDUCE straight from PSUM (2 elem/cycle).
    G-tiles: GpSimd scalar_tensor_tensor min folds PSUM pairs into bf16
             SBUF; DVE finishes with 4x-mode bf16 TSP + one TTR.
    A-tiles: ScalarEngine copies PSUM->bf16 SBUF; DVE folds with 4x-mode
             TSP cascade + one TTR.
  Host takes sqrt and means (tiny: 4096 values/problem).
"""

import sys

sys.path.insert(0, "/opt/trn_rl_repo")

from contextlib import ExitStack

import ml_dtypes
import numpy as np

import concourse.bass as bass
import concourse.tile as tile
from concourse import mybir
from concourse.bass_utils import run_bass_kernel_spmd
from concourse.tile import ScopedClock

BF16 = ml_dtypes.bfloat16

N = 4096          # points per set
K = 24            # split-matmul contraction rows
N_TILES = 32      # 4096 / 128 i-tiles
UNITS = 4         # j-chunks of 1024 per i-tile
BIG = 3.0e38      # min-reduce init / filler

# Per-UNIT evacuation engine, rotating with period 8 over the global unit
# stream.  Each unit's PSUM banks must be freed within 3 PE unit-times
# (~1281ns at full clock) or the PE stalls and drops out of its fast
# p-state.  The rotation spaces every engine's units at least as far apart
# as its per-unit latency: DVE ttr 658ns, Pool TSP 806ns, Act copy 1038ns.
#   D: DVE tensor_tensor_reduce, PSUM pair -> min column directly.
#   P: GpSimd scalar_tensor_tensor, PSUM pair -> 512 bf16 chunk.
#   A: Act copy, PSUM -> 1024 bf16 chunk.
# P=64/A=48/D=16 units saturates Pool at 94%, Act at 91%, DVE at ~100% --
# the capacity-balance optimum.  Chunks land in a per-tile buffer; a
# deferred DVE ttr folds them to the tile's min column, flushed one per
# unit boundary but never just before a D-unit (it would delay the
# PSUM-freeing ttr past its deadline).
UNIT_PAT = ["A", "P", "A", "P", "P", "A", "P", "D"]


def _patch_tile_drain():
    """Walrus (CoreV3) rejects the TileContext tail Drain when it carries >1
    sem wait ("Too many sync wait commands").  Split the waits across
    preceding SP NOPs, one wait each."""
    if getattr(tile.TileContext, "_drain_patched", False):
        return

    def _drain_and_barrier(self, tick_clock, wait_clock):
        nc = self.nc
        nops = [nc.sync.nop() for _ in range(31)]
        drain_inst = nc.sync.drain()
        wait_clock.add_sem_waits(
            drain_inst.ins, ScopedClock({None: tick_clock.global_clock})
        )
        si = drain_inst.ins.sync_info
        waits = list(si.on_wait or [])
        used = 0
        if len(waits) > 1:
            si.on_wait = waits[:1]
            used = len(waits) - 1
            for k, w in enumerate(waits[1:]):
                nsi = nops[k].ins.sync_info
                if nsi is None:
                    nops[k].ins.sync_info = mybir.SyncInfo(on_wait=[w], on_update=[])
                else:
                    nsi.on_wait = (nsi.on_wait or []) + [w]
        # drop the unused filler NOPs (each costs ~25ns of SP decode at drain)
        for spare in nops[used:]:
            for bb in nc.m.functions[0].blocks:
                if spare.ins in bb.instructions:
                    bb.instructions.remove(spare.ins)
                    break
        nc.all_engine_barrier()
        popped = nc._tile_sem_poison_stack.pop()
        assert popped is self._sem_poison
        nc.clear_and_free_semaphores(list(self.sems.allocated().values()))
        nc.all_engine_barrier()

    tile.TileContext._drain_and_barrier = _drain_and_barrier
    tile.TileContext._drain_patched = True


_NC_CACHE = None


def _split_multi_waits(nc):
    """This walrus build allows only 1 sem wait per instruction.  Hoist extra
    waits onto the nearest preceding same-engine instruction with a free wait
    slot (in-order engines: waiting earlier is strictly more conservative)."""
    eng_handles = {
        mybir.EngineType.PE: nc.tensor,
        mybir.EngineType.DVE: nc.vector,
        mybir.EngineType.Activation: nc.scalar,
        mybir.EngineType.Pool: nc.gpsimd,
        mybir.EngineType.SP: nc.sync,
    }
    for bb in nc.m.functions[0].blocks:
        insts = list(bb.instructions)
        for idx, inst in enumerate(insts):
            si = inst.sync_info
            if not si or not si.on_wait or len(si.on_wait) <= 1:
                continue
            waits = list(si.on_wait)
            extra = waits[1:]
            si.on_wait = waits[:1]
            for w in extra:
                # Materialize a NOP on this engine right before `inst` to
                # carry the extra wait — same program point, so semantics
                # are identical.  (nop() appends to the function's last
                # block; move it into place.)
                nop = eng_handles[inst.engine].nop().ins
                nc.m.functions[0].blocks[-1].instructions.remove(nop)
                bb.instructions.insert(bb.instructions.index(inst), nop)
                nop.sync_info = mybir.SyncInfo(on_wait=[w], on_update=[])


def _build_nc():
    global _NC_CACHE
    if _NC_CACHE is not None:
        return _NC_CACHE
    _patch_tile_drain()

    nc = bass.Bass(
        "TRN2",
        target_bir_lowering=False,
        debug=False,
        enable_asserts=False,
        num_devices=8,
    )
    inp_ap = nc.dram_tensor("inp", [K, 2 * N], mybir.dt.bfloat16, kind="ExternalInput").ap()
    mins_ap = nc.dram_tensor("mins", [128, N_TILES * UNITS], mybir.dt.float32, kind="ExternalOutput").ap()

    f32 = mybir.dt.float32
    bf16 = mybir.dt.bfloat16
    mn = mybir.AluOpType.min

    with tile.TileContext(nc) as tc:
        with ExitStack() as ctx:
            consts = ctx.enter_context(tc.tile_pool(name="consts", bufs=1))
            psum = ctx.enter_context(tc.tile_pool(name="psum", bufs=4, space="PSUM"))
            tbpool = ctx.enter_context(tc.tile_pool(name="tbuf", bufs=3))
            scr = ctx.enter_context(tc.tile_pool(name="scratch", bufs=1))
            outp = ctx.enter_context(tc.tile_pool(name="outp", bufs=1))

            inp_sb = consts.tile([K, 2 * N], bf16)
            # first matmul's operands first (b chunk, then tile-0 a block),
            # then the rest; DMA issue pipelines but transfers serialize, so
            # fine-grained leading chunks start the PE ~1.5us earlier.
            # HWDGE on SP keeps descriptor generation off the compute engines.
            nc.sync.dma_start(inp_sb[:, N : N + 512], inp_ap[:, N : N + 512])
            nc.sync.dma_start(inp_sb[:, 0:128], inp_ap[:, 0:128])
            nc.sync.dma_start(inp_sb[:, N + 512 : N + 1024], inp_ap[:, N + 512 : N + 1024])
            nc.sync.dma_start(inp_sb[:, N + 1024 : 2 * N], inp_ap[:, N + 1024 : 2 * N])
            nc.sync.dma_start(inp_sb[:, 128:N], inp_ap[:, 128:N])

            mins_sb = outp.tile([128, N_TILES * UNITS], f32)
            nc.vector.memset(mins_sb[:], BIG)

            sc = scr.tile([128, 1536], bf16)

            pending = []  # deferred tail reduces: (tb, nbytes_used, col)

            def flush_tail():
                tb_, used, col = pending.pop(0)
                half = used // 2
                nc.vector.tensor_tensor_reduce(
                    out=sc[:, 0:half],
                    in0=tb_[:, 0:half],
                    in1=tb_[:, half:used],
                    scale=1.0,
                    scalar=BIG,
                    op0=mn,
                    op1=mn,
                    accum_out=mins_sb[:, col : col + 1],
                )

            for t in range(N_TILES):
                lw = inp_sb[:, t * 128 : (t + 1) * 128]
                tb = tbpool.tile([128, 3072], bf16, name="tb")
                off = 0
                tail_col = None
                for u in range(UNITS):
                    g = 4 * t + u
                    e = UNIT_PAT[g % 8]
                    pt = psum.tile([128, 1024], f32)
                    for h in range(2):
                        j0 = N + u * 1024 + h * 512
                        nc.tensor.matmul(
                            pt[:, h * 512 : (h + 1) * 512],
                            lw,
                            inp_sb[:, j0 : j0 + 512],
                            start=True,
                            stop=True,
                        )
                    if e == "D":
                        nc.vector.tensor_tensor_reduce(
                            out=sc[:, 0:512],
                            in0=pt[:, 0:512],
                            in1=pt[:, 512:1024],
                            scale=1.0,
                            scalar=BIG,
                            op0=mn,
                            op1=mn,
                            accum_out=mins_sb[:, g : g + 1],
                        )
                    elif e == "P":
                        nc.gpsimd.scalar_tensor_tensor(
                            out=tb[:, off : off + 512],
                            in0=pt[:, 0:512],
                            scalar=BIG,
                            in1=pt[:, 512:1024],
                            op0=mn,
                            op1=mn,
                        )
                        if tail_col is None:
                            tail_col = g
                        off += 512
                    else:  # A
                        if g % 8 == 5:
                            # this slot's bank otherwise frees ~330ns past
                            # its deadline; two half-copies let the first
                            # start at half-fill and the pipe drain earlier
                            nc.scalar.copy(
                                out=tb[:, off : off + 512],
                                in_=pt[:, 0:512],
                            )
                            nc.scalar.copy(
                                out=tb[:, off + 512 : off + 1024],
                                in_=pt[:, 512:1024],
                            )
                        else:
                            nc.scalar.copy(
                                out=tb[:, off : off + 1024],
                                in_=pt[:, 0:1024],
                            )
                        if tail_col is None:
                            tail_col = g
                        off += 1024
                    # flush one deferred tail per unit boundary, but only in
                    # slots at least a tail-length ahead of the next D-unit:
                    # the PSUM-freeing D ttr must not queue behind a
                    # ~1.2-1.7us tail
                    if pending and g % 8 in (7, 0, 1, 2):
                        flush_tail()
                if off:
                    pending.append((tb, off, tail_col))
                if t == 17:
                    # first half of the output is final once tile 15's tail
                    # has flushed; overlap its DMA with the back half
                    nc.sync.dma_start(mins_ap[:, 0:64], mins_sb[:, 0:64])
            while pending:
                flush_tail()
            nc.sync.dma_start(mins_ap[:, 64:128], mins_sb[:, 64:128])

    _split_multi_waits(nc)
    _NC_CACHE = nc
    return nc


def _split3(x):
    """x (f32) -> three bf16 parts whose (f32) sum ~= x to ~2^-27 rel."""
    x = x.astype(np.float32)
    h = x.astype(BF16).astype(np.float32)
    r = x - h
    l = r.astype(BF16).astype(np.float32)
    q = (r - l).astype(BF16).astype(np.float32)
    return h, l, q


def _prep_problem(A, B):
    """Build lhsT [K, N] and rhs [K, N] bf16 rows for d2 = |a|^2+|b|^2-2a.b."""
    b2 = (B.astype(np.float64) ** 2).sum(1).astype(np.float32)
    a2 = (A.astype(np.float64) ** 2).sum(1).astype(np.float32)
    b2h, b2l, b2q = _split3(b2)
    a2h, a2l, a2q = _split3(a2)
    ah, al, aq = _split3(A)
    bh, bl, bq = _split3(B)
    ones = np.ones(N, np.float32)
    lhs_rows = [ones, ones, ones, a2h, a2l, a2q]
    rhs_rows = [b2h, b2l, b2q, ones, ones, ones]
    for d in range(3):
        for a_, b_ in (
            (ah[:, d], -2.0 * bh[:, d]),
            (ah[:, d], -2.0 * bl[:, d]),
            (al[:, d], -2.0 * bh[:, d]),
            (al[:, d], -2.0 * bl[:, d]),
            (ah[:, d], -2.0 * bq[:, d]),
            (aq[:, d], -2.0 * bh[:, d]),
        ):
            lhs_rows.append(a_)
            rhs_rows.append(b_)
    lhsT = np.stack(lhs_rows).astype(BF16)
    rhs = np.stack(rhs_rows).astype(BF16)
    return np.concatenate([lhsT, rhs], axis=1)  # [K, 2N]


def _run(data1, data2, trace=False):
    d1 = np.asarray(data1, dtype=np.float32).reshape(8, N, 3)
    d2 = np.asarray(data2, dtype=np.float32).reshape(8, N, 3)
    in_maps = []
    for p in range(8):
        in_maps.append({"inp": _prep_problem(d1[p], d2[p])})
    nc = _build_nc()
    res = run_bass_kernel_spmd(nc, in_maps, core_ids=list(range(8)), trace=trace)

    out = np.zeros(2, np.float64)
    for p in range(8):
        m = res.results[p]["mins"]          # [128, 128]; cols = (tile, unit)
        m = m.reshape(128, N_TILES, UNITS).min(axis=-1)   # [128, 32]
        mflat = m.T.reshape(N).astype(np.float64)
        dd = np.sqrt(np.maximum(mflat, 0.0))
        out[p // 4] += dd.mean() / 4.0
    return out.astype(np.float32), res


def kernel(data1, data2, dim):
    dim = int(dim)
    if dim > 0:
        data1 = np.swapaxes(np.asarray(data1), 0, dim)
        data2 = np.swapaxes(np.asarray(data2), 0, dim)
    out, _ = _run(data1, data2, trace=False)
    return out


def kernel_traced(data1, data2, dim):
    """test.py entry: returns (output, BassKernelResults) with profiling."""
    dim = int(dim)
    if dim > 0:
        data1 = np.swapaxes(np.asarray(data1), 0, dim)
        data2 = np.swapaxes(np.asarray(data2), 0, dim)
    return _run(data1, data2, trace=True)
